# revision 1
# baseline (speedup 1.0000x reference)
"""Trainium2 Bass kernel for the distributed-memory embedding lookup problem.

reference computation (all f32):
    inputs = P[doc_ids] + sum_c W[context_ids]        # [B, D]
    out_vecs = outputs.T[sample_ids]                   # [B, S, D]
    result = einsum("bd,bsd->bs", inputs, out_vecs)    # [B, S]

Strategy: data-parallel over the batch across 8 NeuronCores (2048 rows
each); tables replicated in each core's HBM.  Gathers run as SWDGE
indirect DMAs with one index per partition ([128,1] offset APs — the
HW-validated form; multi-column offset APs gather wrong rows on real
HW).  DVE does the context-sum reduce and the dot products with plain
tensor_tensor / tensor_reduce ops.
"""

import numpy as np

_B = 16384
_C = 8
_S = 10
_D = 128
_N_DOCS = 1_000_000
_N_WORDS = 100_000
_NC = 8
_BPC = _B // _NC        # 2048 batch rows per core
_P = 128                # partitions
_TILES = _BPC // _P     # 16 batch tiles per core
_CHUNK = 4              # batch tiles per pipeline chunk

_nc_cache = {}

LAST_RESULT = None      # BassKernelResults of the most recent run (for test.py)


def _build(n_docs=_N_DOCS, n_words=_N_WORDS, tiles=_TILES, chunk=_CHUNK):
    import concourse.bass as bass
    import concourse.mybir as mybir
    import concourse.tile as tile
    from concourse import bacc

    f32 = mybir.dt.float32
    i32 = mybir.dt.int32

    nc = bacc.Bacc(trn_type="TRN2")
    p_tab = nc.dram_tensor("p_tab", (n_docs, _D), f32, kind="ExternalInput")
    w_tab = nc.dram_tensor("w_tab", (n_words, _D), f32, kind="ExternalInput")
    o_tab = nc.dram_tensor("o_tab", (n_words, _D), f32, kind="ExternalInput")
    doc_idx = nc.dram_tensor("doc_idx", (_P, tiles), i32, kind="ExternalInput")
    ctx_idx = nc.dram_tensor("ctx_idx", (_P, tiles * _C), i32, kind="ExternalInput")
    smp_idx = nc.dram_tensor("smp_idx", (_P, tiles * _S), i32, kind="ExternalInput")
    res = nc.dram_tensor("res", (_P, tiles * _S), f32, kind="ExternalOutput")

    nchunks = tiles // chunk
    with tile.TileContext(nc) as tc:
        with (
            tc.tile_pool(name="idxp", bufs=2) as idxp,
            tc.tile_pool(name="datp", bufs=2) as datp,
            tc.tile_pool(name="resp", bufs=1) as resp,
        ):
            res_sb = resp.tile([_P, tiles * _S], f32)

            # Preload ALL indices once: single-column [128,1] offset slices at
            # any column offset are HW-validated; this removes the per-chunk
            # idx DMAs and their semaphore waits from the Pool stream.
            doc_i = idxp.tile([_P, tiles], i32)
            ctx_i = idxp.tile([_P, tiles * _C], i32)
            smp_i = idxp.tile([_P, tiles * _S], i32)
            nc.sync.dma_start(doc_i[:], doc_idx[:])
            nc.sync.dma_start(ctx_i[:], ctx_idx[:])
            nc.sync.dma_start(smp_i[:], smp_idx[:])

            for ch in range(nchunks):
                p_t = datp.tile([_P, chunk * _D], f32, tag="p")
                w_t = datp.tile([_P, chunk * _C * _D], f32, tag="w")
                o_t = datp.tile([_P, chunk * _S * _D], f32, tag="o")
                # one [128,1]-offset indirect DMA per gathered row-column
                for tt in range(chunk):
                    t = ch * chunk + tt
                    nc.gpsimd.indirect_dma_start(
                        out=p_t[:, tt * _D:(tt + 1) * _D],
                        out_offset=None,
                        in_=p_tab[:],
                        in_offset=bass.IndirectOffsetOnAxis(
                            ap=doc_i[:, t:t + 1], axis=0
                        ),
                    )
                    for c in range(_C):
                        j = tt * _C + c
                        g = t * _C + c
                        nc.gpsimd.indirect_dma_start(
                            out=w_t[:, j * _D:(j + 1) * _D],
                            out_offset=None,
                            in_=w_tab[:],
                            in_offset=bass.IndirectOffsetOnAxis(
                                ap=ctx_i[:, g:g + 1], axis=0
                            ),
                        )
                    for s in range(_S):
                        j = tt * _S + s
                        g = t * _S + s
                        nc.gpsimd.indirect_dma_start(
                            out=o_t[:, j * _D:(j + 1) * _D],
                            out_offset=None,
                            in_=o_tab[:],
                            in_offset=bass.IndirectOffsetOnAxis(
                                ap=smp_i[:, g:g + 1], axis=0
                            ),
                        )

                for tt in range(chunk):
                    t = ch * chunk + tt
                    inp = datp.tile([_P, _D], f32, tag="inp")
                    wv = w_t[:, tt * _C * _D:(tt + 1) * _C * _D].rearrange(
                        "p (c d) -> p d c", c=_C
                    )
                    nc.vector.tensor_reduce(
                        out=inp[:], in_=wv,
                        axis=mybir.AxisListType.X, op=mybir.AluOpType.add,
                    )
                    nc.vector.tensor_add(
                        out=inp[:], in0=inp[:], in1=p_t[:, tt * _D:(tt + 1) * _D]
                    )
                    prod = datp.tile([_P, _S * _D], f32, tag="prod")
                    for s in range(_S):
                        j = tt * _S + s
                        nc.vector.tensor_mul(
                            out=prod[:, s * _D:(s + 1) * _D],
                            in0=o_t[:, j * _D:(j + 1) * _D],
                            in1=inp[:],
                        )
                    pv = prod[:].rearrange("p (s d) -> p s d", s=_S)
                    nc.vector.tensor_reduce(
                        out=res_sb[:, t * _S:(t + 1) * _S], in_=pv,
                        axis=mybir.AxisListType.X, op=mybir.AluOpType.add,
                    )
            nc.sync.dma_start(res[:], res_sb[:])
    nc.compile()
    return nc


def _shard_host(doc_ids, context_ids, sample_ids, tiles=_TILES):
    """Per-core index tensors laid out [128 partitions, ...] so a straight
    DMA lands index (tile t, partition p) at SBUF[p, t]."""
    per_core = []
    bpc = tiles * _P
    ncores = doc_ids.shape[0] // bpc
    for c in range(ncores):
        sl = slice(c * bpc, (c + 1) * bpc)
        d = np.ascontiguousarray(doc_ids[sl].reshape(tiles, _P).T)
        cx = np.ascontiguousarray(
            context_ids[sl].reshape(tiles, _P, _C).transpose(1, 0, 2).reshape(_P, tiles * _C)
        )
        sm = np.ascontiguousarray(
            sample_ids[sl].reshape(tiles, _P, _S).transpose(1, 0, 2).reshape(_P, tiles * _S)
        )
        per_core.append((d, cx, sm))
    return per_core


def kernel(doc_ids, context_ids, sample_ids, paragraph_matrix, word_matrix, outputs):
    global LAST_RESULT
    from concourse.bass_utils import run_bass_kernel_spmd

    doc_ids = np.asarray(doc_ids).astype(np.int32)
    context_ids = np.asarray(context_ids).astype(np.int32)
    sample_ids = np.asarray(sample_ids).astype(np.int32)
    P = np.ascontiguousarray(np.asarray(paragraph_matrix, dtype=np.float32))
    W = np.ascontiguousarray(np.asarray(word_matrix, dtype=np.float32))
    OT = np.ascontiguousarray(np.asarray(outputs, dtype=np.float32).T)  # [N_WORDS, D]

    key = (_N_DOCS, _N_WORDS, _TILES, _CHUNK)
    if key not in _nc_cache:
        _nc_cache[key] = _build()
    nc = _nc_cache[key]

    shards = _shard_host(doc_ids, context_ids, sample_ids)
    in_maps = [
        {"p_tab": P, "w_tab": W, "o_tab": OT, "doc_idx": d, "ctx_idx": cx, "smp_idx": sm}
        for (d, cx, sm) in shards
    ]
    LAST_RESULT = run_bass_kernel_spmd(nc, in_maps, core_ids=list(range(_NC)))

    out = np.empty((_B, _S), dtype=np.float32)
    for c in range(_NC):
        r = LAST_RESULT.results[c]["res"]  # [128, TILES*S]
        out[c * _BPC:(c + 1) * _BPC] = (
            r.reshape(_P, _TILES, _S).transpose(1, 0, 2).reshape(_BPC, _S)
        )
    return out



# revision 14
# speedup vs baseline: 1.0074x; 1.0074x over previous
"""Trainium2 Bass kernel for the distributed-memory embedding lookup problem.

reference computation (all f32):
    inputs = P[doc_ids] + sum_c W[context_ids]        # [B, D]
    out_vecs = outputs.T[sample_ids]                   # [B, S, D]
    result = einsum("bd,bsd->bs", inputs, out_vecs)    # [B, S]

Strategy: data-parallel over the batch across 8 NeuronCores (2048 rows
each); tables replicated in each core's HBM as bf16.

HW-measured facts that shape this kernel (see transcript probes):
  - Every data-dependent row gather on TRN2 goes through Q7 SWDGE
    descriptor generation at ~8.3-8.7 ns/row of *serial* Pool-engine
    time, on every available path (indirect_dma_start ~1.11us per
    128-row op; dma_gather ~7.9ns/idx plus int16-index bucketing that
    forces a sorted-compaction + an extra alignment gather).
  - Multi-column offset APs on indirect_dma_start are a BLOCK gather on
    real HW (only idx[p,0] is read; remaining columns get consecutive
    rows), so 128 random rows per op is the hard limit.
  - Therefore the 304 gather ops (16 P + 128 W + 160 O) x ~1.11us
    ~= 340us of GpSimd time is the floor; everything else must hide
    under it.

This version keeps the batch-ordered [128,1]-offset indirect gathers
(no sorting, no alignment passes) and optimizes what is hideable:
bf16 tables (halves DVE work and DMA bytes), per-tile interleave of
the three gather streams, triple-buffered tiles so the DVE never
stalls the GpSimd stream, and one DVE op chain per 128-row tile.
"""

import numpy as np

_B = 16384
_C = 8
_S = 10
_D = 128
_N_DOCS = 1_000_000
_N_WORDS = 100_000
_NC = 8
_BPC = _B // _NC        # 2048 batch rows per core
_P = 128                # partitions
_TILES = _BPC // _P     # 16 batch tiles per core

_nc_cache = {}

LAST_RESULT = None      # BassKernelResults of the most recent run (for test.py)


def _build():
    import concourse.bass as bass
    import concourse.mybir as mybir
    import concourse.tile as tile
    from concourse import bacc

    f32 = mybir.dt.float32
    bf16 = mybir.dt.bfloat16
    i32 = mybir.dt.int32

    nc = bacc.Bacc(trn_type="TRN2")
    p_tab = nc.dram_tensor("p_tab", (_N_DOCS, _D), bf16, kind="ExternalInput")
    w_tab = nc.dram_tensor("w_tab", (_N_WORDS, _D), bf16, kind="ExternalInput")
    o_tab = nc.dram_tensor("o_tab", (_N_WORDS, _D), bf16, kind="ExternalInput")
    doc_idx = nc.dram_tensor("doc_idx", (_P, _TILES), i32, kind="ExternalInput")
    ctx_idx = nc.dram_tensor("ctx_idx", (_P, _TILES * _C), i32, kind="ExternalInput")
    smp_idx = nc.dram_tensor("smp_idx", (_P, _TILES * _S), i32, kind="ExternalInput")
    res = nc.dram_tensor("res", (_P, _TILES * _S), f32, kind="ExternalOutput")

    with tile.TileContext(nc) as tc:
        with (
            tc.tile_pool(name="idxp", bufs=1) as idxp,
            tc.tile_pool(name="datp", bufs=3) as datp,
            tc.tile_pool(name="resp", bufs=1) as resp,
            nc.allow_low_precision(
                reason="bf16 gathers/dots under 2e-2 tolerance; DVE "
                "accumulates in f32 internally"
            ),
        ):
            res_sb = resp.tile([_P, _TILES * _S], f32)

            doc_i = idxp.tile([_P, _TILES], i32)
            ctx_i = idxp.tile([_P, _TILES * _C], i32)
            smp_i = idxp.tile([_P, _TILES * _S], i32)
            nc.sync.dma_start(doc_i[:], doc_idx[:])
            nc.sync.dma_start(ctx_i[:], ctx_idx[:])
            nc.sync.dma_start(smp_i[:], smp_idx[:])

            for t in range(_TILES):
                p_t = datp.tile([_P, _D], bf16, tag="p")
                w_t = datp.tile([_P, _C, _D], bf16, tag="w")
                o_t = datp.tile([_P, _S, _D], bf16, tag="o")
                nc.gpsimd.indirect_dma_start(
                    out=p_t[:], out_offset=None, in_=p_tab[:],
                    in_offset=bass.IndirectOffsetOnAxis(
                        ap=doc_i[:, t:t + 1], axis=0
                    ),
                )
                for c in range(_C):
                    g = t * _C + c
                    nc.gpsimd.indirect_dma_start(
                        out=w_t[:, c, :], out_offset=None, in_=w_tab[:],
                        in_offset=bass.IndirectOffsetOnAxis(
                            ap=ctx_i[:, g:g + 1], axis=0
                        ),
                    )
                for s in range(_S):
                    g = t * _S + s
                    nc.gpsimd.indirect_dma_start(
                        out=o_t[:, s, :], out_offset=None, in_=o_tab[:],
                        in_offset=bass.IndirectOffsetOnAxis(
                            ap=smp_i[:, g:g + 1], axis=0
                        ),
                    )

                inp = datp.tile([_P, 1, _D], bf16, tag="inp")
                nc.vector.tensor_reduce(
                    out=inp[:, 0, :],
                    in_=w_t[:].rearrange("p c d -> p d c"),
                    axis=mybir.AxisListType.X, op=mybir.AluOpType.add,
                )
                nc.vector.tensor_add(
                    out=inp[:, 0, :], in0=inp[:, 0, :], in1=p_t[:]
                )
                prods = datp.tile([_P, _S, _D], bf16, tag="prod")
                nc.vector.tensor_mul(
                    out=prods[:],
                    in0=o_t[:],
                    in1=inp[:].to_broadcast([_P, _S, _D]),
                )
                nc.vector.tensor_reduce(
                    out=res_sb[:, t * _S:(t + 1) * _S], in_=prods[:],
                    axis=mybir.AxisListType.X, op=mybir.AluOpType.add,
                )
            nc.sync.dma_start(res[:], res_sb[:])
    nc.compile()
    return nc


def _shard_host(doc_ids, context_ids, sample_ids):
    """Per-core index tensors laid out [128 partitions, ...] so a straight
    DMA lands index (tile t, partition p) at SBUF[p, t]."""
    per_core = []
    for c in range(_NC):
        sl = slice(c * _BPC, (c + 1) * _BPC)
        d = np.ascontiguousarray(
            doc_ids[sl].reshape(_TILES, _P).T.astype(np.int32))
        cx = np.ascontiguousarray(
            context_ids[sl].reshape(_TILES, _P, _C)
            .transpose(1, 0, 2).reshape(_P, _TILES * _C).astype(np.int32))
        sm = np.ascontiguousarray(
            sample_ids[sl].reshape(_TILES, _P, _S)
            .transpose(1, 0, 2).reshape(_P, _TILES * _S).astype(np.int32))
        per_core.append((d, cx, sm))
    return per_core


def kernel(doc_ids, context_ids, sample_ids, paragraph_matrix, word_matrix, outputs):
    global LAST_RESULT
    from concourse.bass_utils import run_bass_kernel_spmd
    from ml_dtypes import bfloat16

    doc_ids = np.asarray(doc_ids).astype(np.int64)
    context_ids = np.asarray(context_ids).astype(np.int64)
    sample_ids = np.asarray(sample_ids).astype(np.int64)
    P16 = np.asarray(paragraph_matrix, dtype=np.float32).astype(bfloat16)
    W16 = np.asarray(word_matrix, dtype=np.float32).astype(bfloat16)
    O16 = np.ascontiguousarray(
        np.asarray(outputs, dtype=np.float32).T.astype(bfloat16))

    if "nc" not in _nc_cache:
        _nc_cache["nc"] = _build()
    nc = _nc_cache["nc"]

    shards = _shard_host(doc_ids, context_ids, sample_ids)
    in_maps = [
        {"p_tab": P16, "w_tab": W16, "o_tab": O16,
         "doc_idx": d, "ctx_idx": cx, "smp_idx": sm}
        for (d, cx, sm) in shards
    ]
    LAST_RESULT = run_bass_kernel_spmd(nc, in_maps, core_ids=list(range(_NC)))

    out = np.empty((_B, _S), dtype=np.float32)
    for c in range(_NC):
        r = np.asarray(LAST_RESULT.results[c]["res"])  # [128, TILES*S]
        out[c * _BPC:(c + 1) * _BPC] = (
            r.reshape(_P, _TILES, _S).transpose(1, 0, 2).reshape(_BPC, _S)
        )
    return out


def _selfcheck_sim():
    """Run one core through CoreSim and compare against numpy (with a
    nonzero outputs matrix so every phase is exercised)."""
    from concourse.bass_interp import CoreSim
    from ml_dtypes import bfloat16

    rng = np.random.default_rng(0)
    doc = rng.integers(0, _N_DOCS, _B).astype(np.int64)
    ctx = rng.integers(0, _N_WORDS, (_B, _C)).astype(np.int64)
    smp = rng.integers(0, _N_WORDS, (_B, _S)).astype(np.int64)
    P = rng.standard_normal((_N_DOCS, _D), dtype=np.float32)
    W = rng.standard_normal((_N_WORDS, _D), dtype=np.float32)
    O = rng.standard_normal((_N_WORDS, _D), dtype=np.float32)  # outputs.T

    nc = _build()
    c = 0
    sim = CoreSim(nc)
    d, cx, sm = _shard_host(doc, ctx, smp)[c]
    m = {"p_tab": P.astype(bfloat16), "w_tab": W.astype(bfloat16),
         "o_tab": O.astype(bfloat16), "doc_idx": d, "ctx_idx": cx,
         "smp_idx": sm}
    for name, arr in m.items():
        sim.tensor(name)[:] = arr
    sim.simulate()
    r = np.asarray(sim.tensor("res"))
    actual = r.reshape(_P, _TILES, _S).transpose(1, 0, 2).reshape(_BPC, _S)

    sl = slice(c * _BPC, (c + 1) * _BPC)
    inputs = P[doc[sl]] + W[ctx[sl]].sum(axis=1)
    expected = np.einsum("bd,bsd->bs", inputs, O[smp[sl]])
    err = np.abs(actual - expected).max() / np.abs(expected).max()
    print(f"sim rel err: {err:.4e}")
    assert err < 2e-2, err
    print("SIM CHECK PASSED")


if __name__ == "__main__":
    import sys
    if "--sim" in sys.argv:
        _selfcheck_sim()
